# revision 6
# baseline (speedup 1.0000x reference)
"""Trainium2 Bass kernel for nn_Dense_56779467653682.

Computes out = scale * x @ (2*kernel - 1) where x:[8,2048,4096] f32,
kernel:[4096,4096] bool, scale scalar f32 (= 1/64).

Strategy: data-parallel over the 16384 tokens across 8 NeuronCores
(2048 tokens/core), with the matmul run in fp8-e4m3 *DoubleRow* mode
(2 fp8 weights per PE cell -> 2 MACs/cell/cycle, ~2x the bf16 rate).

Precision: pure e4m3 quantization of x gives ~2.7e-2 rel err (gate is
2e-2), so a partial residual-correction pass is added: for the first
KRES/KT2 fraction of the contraction dim, the quantization residual
r = x - e4m3(x) (itself e4m3, exploiting fp8 subnormals) is accumulated
into the *same* PSUM group using the *same* weight tiles. The output
scale 1/64 is folded into the weights (+-2^-6 is exact in e4m3), so
eviction stays a plain PSUM->SBUF copy. KRES=10/16 gives ~1.6e-2.

Per core: out[2048,4096] f32 = (q8 + r8_prefix)[2048,4096] @ w8[4096,4096]

Device tiling (per core):
  - contraction K=4096 -> 16 DoubleRow k-tiles of 256 (= [128 part, 2])
  - tokens M=2048 -> 16 m-tiles of 128 (PSUM partition, lhsT free dim)
  - features N=4096 -> 8 n-chunks of 512 (PSUM free dim = one bank)
  All 16 q8 m-tiles (+ KRES-deep r8 tiles) stay resident in SBUF
  (~13 MB); w streams once in 2 MB n-chunks (double buffered, 256 KB
  pieces); each output tile accumulates 16+KRES back-to-back DoubleRow
  matmuls in one PSUM bank, is copied to SBUF on the DVE, and DMA'd out.
"""

import numpy as np
import ml_dtypes

BATCH, SEQ, IN_DIM, FEATURES = 8, 2048, 4096, 4096
N_CORES = 8
TOKENS = BATCH * SEQ
TOK_PER_CORE = TOKENS // N_CORES  # 2048
P = 128                           # partitions / tile edge
KT2 = IN_DIM // (2 * P)           # 16 DoubleRow k-tiles (256 each)
KRES = 8                          # residual-corrected k-tiles (of KT2)
MT = TOK_PER_CORE // P            # 16 m-tiles
NF = 512                          # features per n-chunk (one PSUM bank)
NT = FEATURES // NF               # 8 n-chunks

_E4 = ml_dtypes.float8_e4m3       # TRN FP8_EXP4-compatible for |v| <= 240

_cache = {}


def _build_program():
    """Build + compile the per-core Bass/Tile program (SPMD, same on all cores)."""
    import concourse.bacc as bacc
    import concourse.mybir as mybir
    from concourse.tile import TileContext

    nc = bacc.Bacc("TRN2", target_bir_lowering=False, debug=False)

    DR = mybir.MatmulPerfMode.DoubleRow
    F8 = mybir.dt.float8e4

    xs_d = nc.dram_tensor("xs", [MT, P, KT2, 2, P], F8, kind="ExternalInput")
    rs_d = nc.dram_tensor("rs", [MT, P, KRES, 2, P], F8, kind="ExternalInput")
    ws_d = nc.dram_tensor("ws", [NT, P, KT2, 2, NF], F8, kind="ExternalInput")
    out_d = nc.dram_tensor("out", [TOK_PER_CORE, FEATURES], mybir.dt.float32, kind="ExternalOutput")

    KG = 2                 # k-tiles (of 256) per w piece -> 256 KB DMAs
    NSUB = KT2 // KG       # 8 pieces per n-chunk
    WARMUP_MMS = 14        # dummy matmuls to lift HAM to K=8/8 during input DMA

    with TileContext(nc) as tc:
        with (
            tc.tile_pool(name="xpool", bufs=1) as xpool,
            tc.tile_pool(name="rpool", bufs=1) as rpool,
            tc.tile_pool(name="wpool", bufs=2 * NSUB) as wpool,
            tc.tile_pool(name="epool", bufs=4) as epool,
            tc.tile_pool(name="warm", bufs=1) as warm,
            tc.tile_pool(name="psum", bufs=3, space="PSUM") as pp,
            tc.tile_pool(name="psumw", bufs=1, space="PSUM") as ppw,
        ):
            # PE warmup: the HAM clock gate only reaches 2.4 GHz after ~3.4us
            # of sustained PE activity. Burn the initial DMA wait on dummy
            # matmuls so the real ones start at full clock.
            wu = warm.tile([P, 256], mybir.dt.bfloat16, name="wu")
            nc.gpsimd.memset(wu[:], 0.0)
            wups = ppw.tile([P, 256], mybir.dt.float32, name="wups")
            for _ in range(WARMUP_MMS):
                nc.tensor.matmul(wups[:], wu[:, :P], wu[:], start=True, stop=True)

            # Resident q8 tiles [128, KT2, 2, 128] and r8 tiles
            # [128, KRES, 2, 128]; w streams as [128, KG, 2, 512] pieces
            # (256 KB) so matmuls wait on small DMAs. All loads share the
            # sync engine's HWDGE queue: the single FIFO keeps the ramp's
            # arrival order exactly the consumption order.
            w_tiles = [None] * NT

            def w_sub(nt, g):
                wt = wpool.tile(
                    [P, KG, 2, NF], F8, name=f"w_{nt}_{g}", tag="w"
                )
                nc.sync.dma_start(out=wt[:], in_=ws_d[nt, :, g * KG:(g + 1) * KG])
                return wt

            def load_w(nt):
                w_tiles[nt] = [w_sub(nt, g) for g in range(NSUB)]

            def x_tile(mt):
                xt = xpool.tile([P, KT2, 2, P], F8, name=f"xs_t{mt}")
                nc.scalar.dma_start(out=xt[:], in_=xs_d[mt])
                return xt

            def r_tile(mt):
                rt = rpool.tile([P, KRES, 2, P], F8, name=f"rs_t{mt}")
                nc.scalar.dma_start(out=rt[:], in_=rs_d[mt])
                return rt

            # Ramp: first two m-tiles in k-halves (256 KB), interleaved with
            # the first w chunk's pieces in exactly the order the
            # pair-interleaved matmuls below consume them.
            KH = KT2 // 2
            xs_sub = {0: [], 1: []}

            def x_half(mt, h):
                xh = xpool.tile([P, KH, 2, P], F8, name=f"xs_t{mt}_{h}")
                nc.scalar.dma_start(
                    out=xh[:], in_=xs_d[mt, :, h * KH:(h + 1) * KH]
                )
                xs_sub[mt].append(xh)

            # Arrival order tuned against piece-level consumption: mt0 runs
            # solo through w pieces 0-1 (below), so w0[1] is needed before
            # x1's first half.
            x_half(0, 0)
            w0 = [w_sub(0, 0), w_sub(0, 1)]
            x_half(1, 0)
            w0 += [w_sub(0, g) for g in range(2, NSUB // 2)]
            x_half(0, 1)
            x_half(1, 1)
            w0 += [w_sub(0, g) for g in range(NSUB // 2, NSUB)]
            w_tiles[0] = w0

            rs_t = [r_tile(0), r_tile(1)]
            xs_t = [None, None]
            for mt in range(2, MT):
                xs_t.append(x_tile(mt))
                rs_t.append(r_tile(mt))

            def x_slice(mt, kt2):
                if mt < 2:
                    return xs_sub[mt][kt2 // KH][:, kt2 % KH]
                return xs_t[mt][:, kt2]

            def r_slice(mt, kt2):
                return rs_t[mt][:, kt2]

            def w_slice(nt, kt2):
                return w_tiles[nt][kt2 // KG][:, kt2 % KG]

            def mm(ps, lhsT, rhs, start, stop):
                nc.tensor.matmul(
                    ps[:], lhsT, rhs, start=start, stop=stop, perf_mode=DR
                )

            def finish_tile(nt, mt, ps):
                ev = epool.tile([P, NF], mybir.dt.float32, name="ev", tag="ev")
                nc.vector.tensor_copy(ev[:], ps[:])
                nc.gpsimd.dma_start(
                    out=out_d[mt * P:(mt + 1) * P, nt * NF:(nt + 1) * NF],
                    in_=ev[:],
                )

            def tile_mms(nt, mt, ps):
                for kt2 in range(KT2):
                    mm(ps, x_slice(mt, kt2), w_slice(nt, kt2),
                       start=(kt2 == 0), stop=False)
                for kt2 in range(KRES):
                    mm(ps, r_slice(mt, kt2), w_slice(nt, kt2),
                       start=False, stop=(kt2 == KRES - 1))

            for nt in range(NT):
                if w_tiles[nt] is None:
                    load_w(nt)
                if nt == 0:
                    # Ramp: the first w chunk is still streaming in, and the
                    # PE eats one (m-tile, w-piece) block faster than its DMA.
                    # Interleave m-tile pairs (two open PSUM groups) so each
                    # w piece feeds 2x the PE work and the DMA keeps up
                    # from the very first matmul. mt0 runs solo through the
                    # first two pieces (x1's first half lands after w0[1]),
                    # then mt1 catches up and the pair interleaves.
                    for mp in range(0, 4, 2):
                        ps_a = pp.tile([P, 2 * NF], mybir.dt.float32, name="ps", tag="ps")[:, :NF]
                        ps_b = pp.tile([P, 2 * NF], mybir.dt.float32, name="ps2", tag="ps")[:, :NF]
                        if mp == 0:
                            for kt2 in range(2 * KG):
                                mm(ps_a, x_slice(0, kt2), w_slice(0, kt2),
                                   start=(kt2 == 0), stop=False)
                            for kt2 in range(2 * KG):
                                mm(ps_b, x_slice(1, kt2), w_slice(0, kt2),
                                   start=(kt2 == 0), stop=False)
                            g0 = 2
                        else:
                            g0 = 0
                        for g in range(g0, NSUB):
                            for mt, ps in ((mp, ps_a), (mp + 1, ps_b)):
                                for kk in range(KG):
                                    kt2 = g * KG + kk
                                    mm(ps, x_slice(mt, kt2), w_slice(nt, kt2),
                                       start=(kt2 == 0), stop=False)
                        # residual pass, pair-interleaved over the same pieces
                        for g in range((KRES + KG - 1) // KG):
                            for mt, ps in ((mp, ps_a), (mp + 1, ps_b)):
                                for kk in range(KG):
                                    kt2 = g * KG + kk
                                    if kt2 < KRES:
                                        mm(ps, r_slice(mt, kt2), w_slice(nt, kt2),
                                           start=False, stop=(kt2 == KRES - 1))
                        finish_tile(nt, mp, ps_a)
                        finish_tile(nt, mp + 1, ps_b)
                    mts = range(4, MT)
                else:
                    mts = range(MT)
                for mt in mts:
                    ps = pp.tile([P, 2 * NF], mybir.dt.float32, name="ps", tag="ps")[:, :NF]
                    tile_mms(nt, mt, ps)
                    finish_tile(nt, mt, ps)

    nc.compile()
    return nc


def _prep_inputs(x, kern, scale):
    """Host-side: fold scale into ternary e4m3 weights; quantize x to e4m3
    with an e4m3 residual for the first KRES k-tiles; tile per core."""
    s = float(np.asarray(scale))
    # w[k, f] = +-scale = +-2^-6, exact in e4m3.
    w8 = np.where(np.asarray(kern), np.float32(s), np.float32(-s)).astype(_E4)
    # ws[nt, kp, kt2, two, n] = w8[kt2*256 + two*128 + kp, nt*512 + n]
    ws = np.ascontiguousarray(
        w8.reshape(KT2, 2, P, NT, NF).transpose(3, 2, 0, 1, 4)
    )

    xf = np.asarray(x).reshape(TOKENS, IN_DIM)
    in_maps = []
    for c in range(N_CORES):
        xc = np.ascontiguousarray(xf[c * TOK_PER_CORE:(c + 1) * TOK_PER_CORE])
        q8 = xc.astype(_E4)
        r8 = (xc - q8.astype(np.float32)).astype(_E4)
        # xs[mt, kp, kt2, two, mi] = q8[mt*128 + mi, kt2*256 + two*128 + kp]
        xs = np.ascontiguousarray(
            q8.reshape(MT, P, KT2, 2, P).transpose(0, 4, 2, 3, 1)
        )
        rs = np.ascontiguousarray(
            r8.reshape(MT, P, KT2, 2, P).transpose(0, 4, 2, 3, 1)[:, :, :KRES]
        )
        in_maps.append({"xs": xs, "rs": rs, "ws": ws})
    return in_maps


def _ensure_trace_hook():
    """If tracing is requested (e.g. BASS_TRACE=1 in the env) bass_utils
    imports antenv.axon_hooks, which some images lack — that would crash the
    run. Register a functional shim (backed by trn_agent_boot's ctypes hook
    when available) only when the real module is missing, and make the
    artifact upload non-fatal in that degraded environment."""
    import os
    import sys
    import types

    try:
        import antenv.axon_hooks  # noqa: F401
        return
    except ImportError:
        pass
    try:
        import antenv
    except ImportError:
        return
    mod = types.ModuleType("antenv.axon_hooks")
    _state = {"hook": None}
    mod.set_axon_ntff_profile_hook = lambda h: _state.__setitem__("hook", h)
    mod.get_axon_ntff_profile_hook = lambda: _state["hook"]
    sys.modules["antenv.axon_hooks"] = mod
    antenv.axon_hooks = mod
    try:
        from trn_agent_boot.trn_boot import _ntff_profile_via_ctypes

        so = "/opt/axon/libaxon_pjrt.so"
        if os.path.exists(so):
            mod.set_axon_ntff_profile_hook(_ntff_profile_via_ctypes(so))
    except Exception:
        pass
    try:
        from concourse import bass_utils as _bu

        _orig = _bu.upload_artifacts

        def _safe_upload(tmpdir):
            try:
                return _orig(tmpdir)
            except Exception:
                return f"local://{tmpdir}"

        _bu.upload_artifacts = _safe_upload
    except Exception:
        pass


def _run(inputs, trace=False, tmpdir=None):
    from concourse.bass_utils import run_bass_kernel_spmd

    _ensure_trace_hook()

    if "nc" not in _cache:
        _cache["nc"] = _build_program()
    nc = _cache["nc"]

    in_maps = _prep_inputs(inputs["x"], inputs["kernel"], inputs["scale"])
    res = run_bass_kernel_spmd(
        nc, in_maps, core_ids=list(range(N_CORES)), trace=trace, tmpdir=tmpdir
    )
    out = np.concatenate(
        [res.results[c]["out"][None] for c in range(N_CORES)], axis=0
    ).reshape(BATCH, SEQ, FEATURES)
    return np.ascontiguousarray(out.astype(np.float32, copy=False)), res


def kernel(**inputs):
    out, _ = _run(inputs, trace=False)
    return out
